# revision 22
# baseline (speedup 1.0000x reference)
"""3-layer GAT (arXiv-style) on 8 Trainium2 NeuronCores via Bass.

v4: dst-node sharding (6250 nodes/core) with fp8 feature tables.
- Node phase computes table rows [h fp8 | a_src-dot bf16]; table is split in
  two row-halves (A/B) so each AllGathered half stays under 32768 rows,
  letting the edge phase fetch per-edge rows with int16-indexed dma_gather
  (chunks of <=8 subcols, spread over 4 SWDGE queues for parallel Q7
  descriptor generation).
- Per-core node->tile assignment is degree-balanced (host permutation) to
  minimize the padded edge-slot count NSUB.
- Edge phase of layer l is interleaved with node phase of layer l+1 so the
  AllGathers and node compute hide under edge-phase work.
- Segment softmax + aggregation are fp8 DoubleRow PSUM matmuls against 0/1
  selection matrices; exp uses a per-dst shift (cancels in the ratio) to
  stay in fp8 range. Output: log_softmax logits.
"""
import numpy as np
import ml_dtypes

import concourse.bass as bass
import concourse.mybir as mybir
import concourse.tile as tile
from concourse import library_config
from concourse.bass_utils import run_bass_kernel_spmd

# ---- problem constants (hardcoded per harness contract) ----
N = 50000
E = 800000
F_IN = 128
NEG = 0.2
BN_EPS = 1e-5
NC = 8
NS = N // NC            # 6250 nodes per core
NTILE = (NS + 127) // 128   # 49 dst tiles per core
PADN = 13 * 512         # node-phase padded slice rows (6656)
RSPL = 6 * 512          # row split within each core's slice (3072 | 3178)
NTA = RSPL // 128       # 24 dst tiles holding A-half nodes
NA = NC * RSPL          # rows of gathered table half A (24576 < 32768)
NB = NC * (NS - RSPL)   # rows of half B (25424 < 32768)
ROW1 = 768              # L0/L1 table row bytes: h fp8(512) + as bf16(8) + pad
ROW2 = 256              # L2 row: h fp8(240) + as bf16(12) + pad
H12, C12 = 4, 128
H2, C2 = 6, 40
AF = mybir.ActivationFunctionType
ALU = mybir.AluOpType
PM = mybir.MatmulPerfMode
dt = mybir.dt
F32, BF16, I16, F8 = dt.float32, dt.bfloat16, dt.int16, dt.float8e4
bf = ml_dtypes.bfloat16
f8 = ml_dtypes.float8_e4m3


def _hoist_waits(nc, max_keep=1):
    n = 0
    for f in nc.m.functions:
        for bb in f.blocks:
            out, changed = [], False
            for ins in bb.instructions:
                si = getattr(ins, "sync_info", None)
                if si is not None and si.on_wait:
                    keep = 0 if (isinstance(ins, mybir.InstDMACopy)
                                 and getattr(ins, "queue", None) == "qPoolDynamic") else max_keep
                    waits = list(si.on_wait)
                    if len(waits) > keep:
                        cut = len(waits) - keep
                        for w in waits[:cut]:
                            out.append(mybir.InstEventSemaphore(
                                name=f"I-hw-{n}", engine=ins.engine, ins=[], outs=[],
                                sync_info=mybir.SyncInfo(on_wait=[w], on_update=[])))
                            n += 1
                        si.on_wait = waits[cut:]
                        changed = True
                out.append(ins)
            if changed:
                bb.instructions = out
    return n


class _Ctx:
    """Shared per-build state threaded through phase emitters."""
    pass


def _edge_tiles(cx, lay, tiles):
    """Emit edge-phase work for the given dst tiles of layer `lay`."""
    nc, sb, ps = cx.nc, cx.sb, cx.ps
    NSA, NSB = cx.NSA, cx.NSB
    NSUB = NSA + NSB
    ROW = ROW1 if lay < 2 else ROW2
    H = H12 if lay < 2 else H2
    C = C12 if lay < 2 else C2
    HC = H * C
    ASOFF = 512 if lay < 2 else 240
    tabA, tabB = cx.tabs[lay]
    adsl = cx.adsls[lay]
    pout_next = cx.pouts[lay] if lay < 2 else None
    for t in tiles:
        nreal = min(128, NS - t * 128)
        it = sb.tile([128, NSUB * 8], I16, tag="eidx")
        nc.sync.dma_start(out=it[:], in_=cx.eidx[t])
        dl = sb.tile([128, NSUB], BF16, tag="edl")
        nc.sync.dma_start(out=dl[:], in_=cx.edl[t])
        dlr = sb.tile([128, NSUB * 128], BF16, tag="edlr")
        nc.scalar.dma_start(out=dlr[:], in_=cx.edlr[t:t + 1, :].to_broadcast([128, NSUB * 128]))
        adt = sb.tile([128, 8], F8, tag="eadt")
        nc.sync.dma_start(out=adt[:, :H], in_=adsl[t * 128:(t + 1) * 128, :H])

        G = sb.tile([128, NSUB * ROW], F8, tag="eG")
        # SWDGE: <=1024 descriptors per gather; 2 chunks per half spread
        # over the 4 SWDGE queues (Q7 desc-gen runs in parallel).
        qn = 0
        for tab, s0, nsg in ((tabA, 0, NSA), (tabB, NSA, NSB)):
            c0 = 0
            while c0 < nsg:
                ck = min((nsg + 1) // 2, 8, nsg - c0)
                nc.gpsimd.dma_gather(
                    out_ap=G[:, (s0 + c0) * ROW:(s0 + c0 + ck) * ROW]
                        .rearrange("p (n e) -> p n e", e=ROW),
                    in_ap=tab[:, :], idxs_ap=it[:, (s0 + c0) * 8:(s0 + c0 + ck) * 8],
                    num_idxs=ck * 128, num_idxs_reg=cx.regs[ck * 128], elem_size=ROW,
                    queue_num=qn % 4)
                qn += 1
                c0 += ck
        Gr = G[:].rearrange("p (n e) -> p n e", e=ROW)
        Gas = Gr[:, :, ASOFF:ASOFF + 2 * H].bitcast(BF16)    # [128, NSUB, H]

        S = sb.tile([128, NSUB * 128], F8, tag="eS")
        nc.vector.tensor_tensor(
            out=S[:].rearrange("p (n f) -> p n f", f=128),
            in0=dl[:].rearrange("p (n o) -> p n o", o=1).to_broadcast([128, NSUB, 128]),
            in1=cx.io[:].rearrange("p (o f) -> p o f", o=1).to_broadcast([128, NSUB, 128]),
            op=ALU.is_equal)
        ST = sb.tile([128, NSUB * 128], F8, tag="eST")
        nc.vector.tensor_tensor(
            out=ST[:], in0=dlr[:], in1=cx.ioc[:].to_broadcast([128, NSUB * 128]),
            op=ALU.is_equal)

        pad_ = ps.tile([128, NSUB * H], F32, space="PSUM", tag="pad")
        for j in range(NSUB):
            nc.tensor.matmul(out=pad_[:, H * j:H * (j + 1)],
                             lhsT=ST[:, j * 128:(j + 1) * 128],
                             rhs=adt[:, :H], start=True, stop=True)

        # w = exp(leaky(as+ad) - ad - 3) = exp(as - 0.8*min(as+ad,0) - 3)
        # (per-dst shift ad+3 cancels in num/den ratio; keeps fp8 in range)
        sE = sb.tile([128, NSUB * H], F32, tag="esE")
        nc.vector.tensor_tensor(out=sE[:].rearrange("p (n h) -> p n h", h=H),
                                in0=Gas, in1=pad_[:].rearrange("p (n h) -> p n h", h=H),
                                op=ALU.add)
        tS = sb.tile([128, NSUB * H], F32, tag="etS")
        nc.vector.tensor_scalar(out=tS[:], in0=sE[:], scalar1=0.0, scalar2=-0.8,
                                op0=ALU.min, op1=ALU.mult)
        warg = sb.tile([128, NSUB * H], F32, tag="ewarg")
        nc.vector.tensor_tensor(out=warg[:].rearrange("p (n h) -> p n h", h=H),
                                in0=Gas, in1=tS[:].rearrange("p (n h) -> p n h", h=H),
                                op=ALU.add)
        wb = sb.tile([128, NSUB * H], F8, tag="ewb")
        nc.scalar.activation(out=wb[:], in_=warg[:], func=AF.Exp, bias=cx.m3[:, 0:1])

        Gw = sb.tile([128, NSUB * HC], F8, tag="eGw")
        nc.vector.tensor_tensor(
            out=Gw[:].rearrange("p (n h c) -> p n h c", h=H, c=C),
            in0=Gr[:, :, 0:HC].rearrange("p n (h c) -> p n h c", h=H),
            in1=wb[:].rearrange("p (n h o) -> p n h o", h=H, o=1).to_broadcast([128, NSUB, H, C]),
            op=ALU.mult)

        pnum = ps.tile([128, HC], F32, space="PSUM", tag="pnum")
        pden = ps.tile([128, H], F32, space="PSUM", tag="pden")
        NP = NSUB // 2
        for jp in range(NP):
            lhsT = S[:, jp * 256:(jp + 1) * 256].rearrange("p (two f) -> p two f", two=2)
            nc.tensor.matmul(out=pnum[:], lhsT=lhsT,
                             rhs=Gw[:, jp * 2 * HC:(jp + 1) * 2 * HC]
                                 .rearrange("p (two f) -> p two f", two=2),
                             start=(jp == 0), stop=(jp == NP - 1),
                             perf_mode=PM.DoubleRow)
            nc.tensor.matmul(out=pden[:], lhsT=lhsT,
                             rhs=wb[:, jp * 2 * H:(jp + 1) * 2 * H]
                                 .rearrange("p (two f) -> p two f", two=2),
                             start=(jp == 0), stop=(jp == NP - 1),
                             perf_mode=PM.DoubleRow)
        rden = sb.tile([128, H], F32, tag="erden")
        nc.vector.reciprocal(out=rden[:], in_=pden[:])

        if lay < 2:
            ot = sb.tile([128, 512], BF16, tag="eot")
            nc.vector.tensor_tensor(
                out=ot[:].rearrange("p (h c) -> p h c", h=H),
                in0=pnum[:].rearrange("p (h c) -> p h c", h=H),
                in1=rden[:].rearrange("p (h o) -> p h o", o=1).to_broadcast([128, H, C]),
                op=ALU.mult)
            nc.scalar.dma_start(out=pout_next[t * 128:t * 128 + nreal, :], in_=ot[:nreal, :])
        else:
            tmp = sb.tile([128, 240], F32, tag="etmp")
            nc.vector.tensor_tensor(
                out=tmp[:].rearrange("p (h c) -> p h c", h=H),
                in0=pnum[:].rearrange("p (h c) -> p h c", h=H),
                in1=rden[:].rearrange("p (h o) -> p h o", o=1).to_broadcast([128, H, C]),
                op=ALU.mult)
            acc = sb.tile([128, 40], F32, tag="eacc")
            nc.vector.tensor_add(out=acc[:], in0=tmp[:, 0:40], in1=tmp[:, 40:80])
            for hh in range(2, 6):
                nc.vector.tensor_add(out=acc[:], in0=acc[:], in1=tmp[:, hh * 40:(hh + 1) * 40])
            z = sb.tile([128, 40], F32, tag="ez")
            nc.vector.tensor_scalar_mul(out=z[:], in0=acc[:], scalar1=1.0 / 6.0)
            nc.vector.tensor_add(out=z[:], in0=z[:], in1=cx.b2[:])
            mx = sb.tile([128, 1], F32, tag="emx")
            nc.vector.reduce_max(out=mx[:], in_=z[:], axis=mybir.AxisListType.X)
            zs = sb.tile([128, 40], F32, tag="ezs")
            nc.vector.tensor_scalar(out=zs[:], in0=z[:], scalar1=mx[:, 0:1], scalar2=None,
                                    op0=ALU.subtract)
            ex = sb.tile([128, 40], F32, tag="eex")
            nc.scalar.activation(out=ex[:], in_=zs[:], func=AF.Exp)
            sm = sb.tile([128, 1], F32, tag="esm")
            nc.vector.reduce_sum(out=sm[:], in_=ex[:], axis=mybir.AxisListType.X)
            ln = sb.tile([128, 1], F32, tag="eln")
            nc.scalar.activation(out=ln[:], in_=sm[:], func=AF.Ln)
            lsm = sb.tile([128, 40], F32, tag="elsm")
            nc.vector.tensor_scalar(out=lsm[:], in0=zs[:], scalar1=ln[:, 0:1], scalar2=None,
                                    op0=ALU.subtract)
            nc.scalar.dma_start(out=cx.out[t * 128:t * 128 + nreal, :], in_=lsm[:nreal, :])


def _node_its(cx, lay, its):
    """Emit node-phase work for the given 512-row blocks of layer `lay`.
    AllGather A fires after block 5, AllGather B after block 12."""
    nc, sb, psn = cx.nc, cx.sb, cx.psn
    if lay == 0:
        d_in, d_out, H = 128, 512, 4
        wt, blk, bn_sc, bn_sh, pout = cx.w0, 520, None, None, None
    elif lay == 1:
        d_in, d_out, H = 512, 512, 4
        wt, blk, bn_sc, bn_sh, pout = cx.w1, 520, cx.sc0, cx.sh0, cx.pouts[0]
    else:
        d_in, d_out, H = 512, 256, 6
        wt, blk, bn_sc, bn_sh, pout = cx.w2, 268, cx.sc1, cx.sh1, cx.pouts[1]
    ASOFF = 512 if lay < 2 else 240
    tblA, tblB = cx.tbls[lay]
    tabA, tabB = cx.tabs[lay]
    adsl = cx.adsls[lay]
    NFB = d_in // 128
    for it in its:
        n0 = it * 512
        aT = []
        for fb in range(NFB):
            if lay == 0:
                aT.append(cx.xt[:, n0:n0 + 512])
                continue
            a = sb.tile([128, 512], BF16, tag=f"naT{fb}")
            zb = sb.tile([128, 512], BF16, tag="nzb")
            nc.sync.dma_start(out=zb[:],
                              in_=pout[n0:n0 + 512, fb * 128:(fb + 1) * 128],
                              transpose=True)
            z = sb.tile([128, 512], F32, tag="nzf")
            nc.scalar.activation(out=z[:], in_=zb[:], func=AF.Identity,
                                 bias=bn_sh[:, fb:fb + 1], scale=bn_sc[:, fb:fb + 1])
            mm = sb.tile([128, 512], F32, tag="nmm")
            nc.vector.tensor_scalar_min(out=mm[:], in0=z[:], scalar1=0.0)
            ee = sb.tile([128, 512], F32, tag="nee")
            nc.scalar.activation(out=ee[:], in_=mm[:], func=AF.Exp)
            rr = sb.tile([128, 512], F32, tag="nrr")
            nc.vector.tensor_scalar_max(out=rr[:], in0=z[:], scalar1=0.0)
            nc.vector.tensor_add(out=rr[:], in0=rr[:], in1=ee[:])
            nc.vector.tensor_scalar(out=a[:], in0=rr[:], scalar1=-1.0, scalar2=None,
                                    op0=ALU.add)
            aT.append(a[:])
        for nb in range(4):
            ph = psn.tile([128, d_out], F32, space="PSUM", tag="nph")
            pa = psn.tile([128, 2 * H], F32, space="PSUM", tag="npa")
            for fb in range(NFB):
                lhsT = aT[fb][:, nb * 128:(nb + 1) * 128]
                nc.tensor.matmul(out=ph[:], lhsT=lhsT,
                                 rhs=wt[:, fb * blk:fb * blk + d_out],
                                 start=(fb == 0), stop=(fb == NFB - 1))
                nc.tensor.matmul(out=pa[:], lhsT=lhsT,
                                 rhs=wt[:, fb * blk + d_out:(fb + 1) * blk],
                                 start=(fb == 0), stop=(fb == NFB - 1))
            row0 = n0 + nb * 128
            nreal = min(128, max(0, NS - row0))
            if nreal == 0:
                continue
            tb = sb.tile([128, ASOFF + 2 * H], F8, tag="ntb")
            if lay < 2:
                nc.vector.tensor_copy(out=tb[:, 0:512], in_=ph[:])
            else:
                nc.vector.tensor_copy(out=tb[:, 0:120], in_=ph[:, 0:120])
                nc.vector.tensor_copy(out=tb[:, 120:240], in_=ph[:, 128:248])
            nc.vector.tensor_copy(
                out=tb[:, ASOFF:ASOFF + 2 * H].bitcast(BF16).rearrange("p (n o) -> p n o", o=1),
                in_=pa[:].rearrange("p (n s) -> p n s", s=2)[:, :, 0:1])
            if row0 < RSPL:
                nc.sync.dma_start(out=tblA[row0:row0 + nreal, 0:ASOFF + 2 * H],
                                  in_=tb[:nreal, :])
            else:
                nc.sync.dma_start(out=tblB[row0 - RSPL:row0 - RSPL + nreal, 0:ASOFF + 2 * H],
                                  in_=tb[:nreal, :])
            ab = sb.tile([128, 8], F8, tag="nab")
            nc.vector.tensor_copy(out=ab[:, :H].rearrange("p (n o) -> p n o", o=1),
                                  in_=pa[:].rearrange("p (n s) -> p n s", s=2)[:, :, 1:2])
            nc.sync.dma_start(out=adsl[row0:row0 + nreal, :H], in_=ab[:nreal, :H])
        if it == 5:
            nc.gpsimd.collective_compute("AllGather", ALU.bypass, replica_groups=cx.rg,
                                         ins=[tblA[:]], outs=[tabA[:]])
        if it == 12:
            nc.gpsimd.collective_compute("AllGather", ALU.bypass, replica_groups=cx.rg,
                                         ins=[tblB[:]], outs=[tabB[:]])


def _build(NSA, NSB):
    NSUB = NSA + NSB
    nc = bass.Bass(num_swdge_queues=4)
    cx = _Ctx()
    cx.nc = nc
    cx.NSA, cx.NSB = NSA, NSB
    # ---- inputs (per-core) ----
    xT = nc.declare_dram_parameter("xT", [128, PADN], BF16, isOutput=False)
    W0 = nc.declare_dram_parameter("W0p", [128, 512 + 8], BF16, isOutput=False)
    W1 = nc.declare_dram_parameter("W1p", [128, 4 * (512 + 8)], BF16, isOutput=False)
    W2 = nc.declare_dram_parameter("W2p", [128, 4 * (256 + 12)], BF16, isOutput=False)
    bnsc0 = nc.declare_dram_parameter("bnsc0", [128, 4], F32, isOutput=False)
    bnsh0 = nc.declare_dram_parameter("bnsh0", [128, 4], F32, isOutput=False)
    bnsc1 = nc.declare_dram_parameter("bnsc1", [128, 4], F32, isOutput=False)
    bnsh1 = nc.declare_dram_parameter("bnsh1", [128, 4], F32, isOutput=False)
    b2r = nc.declare_dram_parameter("b2r", [128, 40], F32, isOutput=False)
    iota = nc.declare_dram_parameter("iota", [128, 128], BF16, isOutput=False)
    iotac = nc.declare_dram_parameter("iotac", [128, 1], BF16, isOutput=False)
    eidx = nc.declare_dram_parameter("eidx", [NTILE, 128, NSUB * 8], I16, isOutput=False)
    edl = nc.declare_dram_parameter("edl", [NTILE, 128, NSUB], BF16, isOutput=False)
    edlr = nc.declare_dram_parameter("edlr", [NTILE, NSUB * 128], BF16, isOutput=False)
    out = nc.declare_dram_parameter("out", [NS, 40], F32, isOutput=True)
    cx.eidx, cx.edl, cx.edlr, cx.out = eidx, edl, edlr, out
    # ---- internal ----
    cx.tbls, cx.tabs, cx.adsls, cx.pouts = [], [], [], []
    for l, ROW in enumerate((ROW1, ROW1, ROW2)):
        tblA = nc.dram_tensor(f"tblA{l}", [RSPL, ROW], F8)
        tblB = nc.dram_tensor(f"tblB{l}", [NS - RSPL, ROW], F8)
        tabA = nc.dram_tensor(f"tabA{l}", [NA, ROW], F8, addr_space="Shared")
        tabB = nc.dram_tensor(f"tabB{l}", [NB, ROW], F8, addr_space="Shared")
        cx.tbls.append((tblA, tblB))
        cx.tabs.append((tabA, tabB))
        cx.adsls.append(nc.dram_tensor(f"adsl{l}", [NTILE * 128, 8], F8))
    cx.pouts = [nc.dram_tensor("pout0", [PADN, 512], BF16),
                nc.dram_tensor("pout1", [PADN, 512], BF16)]
    cx.rg = [list(range(NC))]

    with tile.TileContext(nc) as tc:
        with tc.tile_pool(name="cst", bufs=1) as cst, \
             tc.tile_pool(name="sb", bufs=2) as sb, \
             tc.tile_pool(name="ps", bufs=2, space="PSUM") as ps, \
             tc.tile_pool(name="psn", bufs=1, space="PSUM") as psn:
            cx.sb, cx.ps, cx.psn = sb, ps, psn
            io = cst.tile([128, 128], BF16)
            nc.sync.dma_start(out=io[:], in_=iota[:])
            ioc = cst.tile([128, 1], BF16)
            nc.sync.dma_start(out=ioc[:], in_=iotac[:])
            b2 = cst.tile([128, 40], F32)
            nc.sync.dma_start(out=b2[:], in_=b2r[:])
            xt = cst.tile([128, PADN], BF16)
            nc.sync.dma_start(out=xt[:], in_=xT[:])
            w0 = cst.tile([128, 520], BF16)
            nc.sync.dma_start(out=w0[:], in_=W0[:])
            w1 = cst.tile([128, 4 * 520], BF16)
            nc.sync.dma_start(out=w1[:], in_=W1[:])
            w2 = cst.tile([128, 4 * 268], BF16)
            nc.sync.dma_start(out=w2[:], in_=W2[:])
            sc0 = cst.tile([128, 4], F32); nc.sync.dma_start(out=sc0[:], in_=bnsc0[:])
            sh0 = cst.tile([128, 4], F32); nc.sync.dma_start(out=sh0[:], in_=bnsh0[:])
            sc1 = cst.tile([128, 4], F32); nc.sync.dma_start(out=sc1[:], in_=bnsc1[:])
            sh1 = cst.tile([128, 4], F32); nc.sync.dma_start(out=sh1[:], in_=bnsh1[:])
            zz = cst.tile([128, 512], BF16)
            nc.gpsimd.memset(zz[:], 0.0)
            m3 = cst.tile([128, 1], F32)
            nc.gpsimd.memset(m3[:], -3.0)
            nc.gpsimd.load_library(library_config.mlp)  # Q7 dma_gather ucode
            regs = {}
            for nsg in (NSA, NSB):
                c0 = 0
                while c0 < nsg:
                    ck = min((nsg + 1) // 2, 8, nsg - c0)
                    if ck * 128 not in regs:
                        regs[ck * 128] = nc.gpsimd.to_reg(ck * 128)
                    c0 += ck
            cx.io, cx.ioc, cx.b2, cx.xt = io[:], ioc[:], b2[:], xt[:]
            cx.w0, cx.w1, cx.w2 = w0[:], w1[:], w2[:]
            cx.sc0, cx.sh0, cx.sc1, cx.sh1 = sc0[:], sh0[:], sc1[:], sh1[:]
            cx.m3, cx.regs = m3, regs
            # zero pad rows of pout0/pout1 (rows NS..PADN)
            for pout in cx.pouts:
                r = NS
                while r < PADN:
                    k = min(128, PADN - r)
                    nc.sync.dma_start(out=pout[r:r + k, :], in_=zz[:k, :])
                    r += k
            for adsl in cx.adsls:
                nc.sync.dma_start(out=adsl[NS:NTILE * 128, :],
                                  in_=zz[:NTILE * 128 - NS, :8].bitcast(F8)[:, :8])

            # Layer 0 node phase (incl. its AllGathers)
            _node_its(cx, 0, range(13))
            # Edge(l) interleaved with Node(l+1): node-it k consumes pout
            # rows written by edge tiles 4k..4k+3 just before it.
            for l in (0, 1):
                for k in range(13):
                    _edge_tiles(cx, l, range(4 * k, min(4 * k + 4, NTILE)))
                    _node_its(cx, l + 1, [k])
            _edge_tiles(cx, 2, range(NTILE))
    _hoist_waits(nc)
    mybir.codegen_inst_isa_subclasses(nc)  # lower load_library pseudo-instr
    return nc


def _prep_edges(edge_index):
    """Degree-balanced per-core node permutation + edge slot arrays.

    Returns (NSA, NSB, per-core (idx16, dl, dlr) arrays, per-core inverse
    permutation invpos [NS] mapping position -> original local id)."""
    src = edge_index[0].astype(np.int64)
    dst = edge_index[1].astype(np.int64)
    loops = np.arange(N, dtype=np.int64)
    src = np.concatenate([src, loops])
    dst = np.concatenate([dst, loops])
    s_in_a = (src % NS) < RSPL
    d_core = dst // NS
    d_r = dst % NS

    # per-node (core, local r) in-degree split by src half
    degA = np.zeros((NC, NS), np.int64)
    degB = np.zeros((NC, NS), np.int64)
    np.add.at(degA, (d_core[s_in_a], d_r[s_in_a]), 1)
    np.add.at(degB, (d_core[~s_in_a], d_r[~s_in_a]), 1)

    # balanced assignment of nodes to tiles (A-nodes -> tiles 0..23,
    # B-nodes -> tiles 24..48), minimizing max per-tile edge count.
    pos = np.zeros((NC, NS), np.int64)      # node r -> position
    invpos = np.zeros((NC, NS), np.int64)   # position -> node r
    for c in range(NC):
        for grp_r0, grp_r1, t0, ntl in ((0, RSPL, 0, NTA), (RSPL, NS, NTA, NTILE - NTA)):
            rs = np.arange(grp_r0, grp_r1)
            da, db = degA[c, rs], degB[c, rs]
            order = np.argsort(-(da + db), kind="stable")
            cap = np.full(ntl, 128)
            cap[-1] = len(rs) - 128 * (ntl - 1)
            sA = np.zeros(ntl); sB = np.zeros(ntl); cnt = np.zeros(ntl, np.int64)
            fill = [[] for _ in range(ntl)]
            avgA = max(da.sum() / ntl, 1.0); avgB = max(db.sum() / ntl, 1.0)
            for i in order:
                cost = np.maximum((sA + da[i]) / avgA, (sB + db[i]) / avgB)
                cost[cnt >= cap] = np.inf
                b = int(np.argmin(cost))
                fill[b].append(rs[i])
                sA[b] += da[i]; sB[b] += db[i]; cnt[b] += 1
            for b in range(ntl):
                base = (t0 + b) * 128
                for j, r in enumerate(fill[b]):
                    pos[c, r] = base + j
                    invpos[c, base + j] = r
    # NOTE: B group of each core has 3178 nodes over 25 tiles (3200 slots);
    # the last 22 positions (6250..6271 within tile 48) stay empty.

    # edge slot arrays using permuted positions
    s_core = src // NS
    s_pos = pos[s_core, src % NS]
    rowA = s_core * RSPL + s_pos
    rowB = s_core * (NS - RSPL) + (s_pos - RSPL)
    d_pos = pos[d_core, d_r]
    tloc = d_pos // 128
    dloc = d_pos % 128

    per_core = []
    maxA = maxB = 0
    for c in range(NC):
        m = d_core == c
        tiles = []
        for t in range(NTILE):
            mt = m & (tloc == t)
            ia = mt & s_in_a
            ib = mt & ~s_in_a
            tiles.append((rowA[ia], dloc[ia], rowB[ib], dloc[ib]))
            maxA = max(maxA, (ia.sum() + 127) // 128)
            maxB = max(maxB, (ib.sum() + 127) // 128)
        per_core.append(tiles)
    NSA, NSB = int(maxA), int(maxB)
    if (NSA + NSB) % 2:
        NSB += 1
    NSUB = NSA + NSB
    arrs = []
    for c in range(NC):
        idx16 = np.zeros((NTILE, 128, NSUB * 8), np.int16)
        dlf = np.full((NTILE, NSUB * 128), 200.0, np.float32)
        for t, (ra, dla, rb, dlb) in enumerate(per_core[c]):
            for (rows, dls, off, nsg) in ((ra, dla, 0, NSA), (rb, dlb, NSA, NSB)):
                n = len(rows)
                ilist = np.zeros(nsg * 128, np.int64)
                ilist[:n] = rows
                wrap = ilist.reshape(-1, 16).T.astype(np.int16)  # [16, nsg*8]
                idx16[t, :, off * 8:(off + nsg) * 8] = np.tile(wrap, (8, 1))
                s0 = off * 128
                dlf[t, s0:s0 + n] = dls
        dl = dlf.reshape(NTILE, NSUB, 128).transpose(0, 2, 1).copy()
        arrs.append((idx16, dl.astype(bf), dlf.astype(bf)))
    return NSA, NSB, arrs, invpos


def kernel(x, edge_index, W0, as0, ad0, b0, g0, bt0, m0, v0,
           W1, as1, ad1, b1, g1, bt1, m1, v1,
           W2, as2, ad2, b2):
    x = np.asarray(x, np.float32)
    NSA, NSB, earrs, invpos = _prep_edges(np.asarray(edge_index))

    # host-side packing
    def packW(W, a_s, a_d, d_out_pad, Hh, Cc, headpad):
        d_in = W.shape[0]
        NFB = d_in // 128
        if headpad:  # L2: pad head layout (3 heads of 40 per 128-block)
            Wp = np.zeros((d_in, 256), np.float32)
            Wp[:, 0:120] = W[:, 0:120]
            Wp[:, 128:248] = W[:, 120:240]
        else:
            Wp = W.astype(np.float32)
        Wa = np.zeros((d_in, 2 * Hh), np.float32)
        for h in range(Hh):
            Wa[:, 2 * h] = W[:, h * Cc:(h + 1) * Cc] @ a_s[h]
            Wa[:, 2 * h + 1] = W[:, h * Cc:(h + 1) * Cc] @ a_d[h]
        blk = np.concatenate([Wp, Wa], axis=1)
        blk = blk.reshape(NFB, 128, -1)
        return np.concatenate([blk[i] for i in range(NFB)], axis=1).astype(bf)

    W0 = np.asarray(W0, np.float32); W1 = np.asarray(W1, np.float32); W2 = np.asarray(W2, np.float32)
    as0 = np.asarray(as0, np.float32); ad0 = np.asarray(ad0, np.float32)
    as1 = np.asarray(as1, np.float32); ad1 = np.asarray(ad1, np.float32)
    as2 = np.asarray(as2, np.float32); ad2 = np.asarray(ad2, np.float32)
    W0p = packW(W0, as0, ad0, 512, 4, 128, False)
    W1p = packW(W1, as1, ad1, 512, 4, 128, False)
    W2p = packW(W2, as2, ad2, 256, 6, 40, True)

    def bnfold(g, bt, m, v, b):
        sc = np.asarray(g, np.float32) / np.sqrt(np.asarray(v, np.float32) + BN_EPS)
        sh = (np.asarray(b, np.float32) - np.asarray(m, np.float32)) * sc + np.asarray(bt, np.float32)
        return sc.reshape(4, 128).T.copy(), sh.reshape(4, 128).T.copy()
    sc0, sh0 = bnfold(g0, bt0, m0, v0, b0)
    sc1, sh1 = bnfold(g1, bt1, m1, v1, b1)
    b2rep = np.broadcast_to(np.asarray(b2, np.float32), (128, 40)).copy()

    iota = np.broadcast_to(np.arange(128, dtype=np.float32).astype(bf), (128, 128)).copy()
    iotac = np.arange(128, dtype=np.float32).astype(bf)[:, None].copy()

    xT = x.T.astype(bf)   # [128, N]
    in_maps = []
    for c in range(NC):
        xts = np.zeros((128, PADN), bf)
        xts[:, :NS] = xT[:, c * NS + invpos[c]]   # permuted node order
        idx16, dl, dlr = earrs[c]
        in_maps.append({
            "xT": xts, "W0p": W0p, "W1p": W1p, "W2p": W2p,
            "bnsc0": sc0, "bnsh0": sh0, "bnsc1": sc1, "bnsh1": sh1,
            "b2r": b2rep, "iota": iota, "iotac": iotac,
            "eidx": idx16, "edl": dl, "edlr": dlr,
        })

    global _last_in_maps, _last_args
    _last_in_maps = in_maps
    _last_args = (NSA, NSB)
    nc = _build(NSA, NSB)
    res = run_bass_kernel_spmd(nc, in_maps, core_ids=list(range(NC)))
    out = np.empty((N, 40), np.float32)
    for c in range(NC):
        out[c * NS + invpos[c]] = res.results[c]["out"].astype(np.float32)
    return out
